# Initial kernel scaffold
#
"""Causal single-head attention (B=4, S=4096, E=1024, H=128) on 8 TRN2 NeuronCores.

Sharding: 8 cores = 4 batches x 2 sequence shards. Each core handles 4 query
blocks of 512 rows of one batch. Causal work per q-block j is 4*(j+1) k-tiles
(128 keys each); blocks are split {7,5,2,0} / {6,4,3,1} so both shards cost 72
k-tiles, padded to a uniform program of [32,24,16,8] k-tiles per slot so all 8
cores run one SPMD program. Per-core data (gathered Q columns + causal masks)
encodes which q-blocks a core owns.

Per core: project Q^T, K^T, V^T from embds^T (bf16 matmuls, fp32 PSUM),
transpose V^T->V on the PE, then flash-style attention in scores-transposed
layout (scores^T = K_tile^T.T @ Q^T), exp on ACT, multiplicative causal masks,
AV accumulated in PSUM over k-tiles, and ones-matmul row-sums for the softmax
normalizer (reciprocal + DMA partition-broadcast at the end).
"""

import numpy as np
import ml_dtypes

import concourse.bacc as bacc
import concourse.bass as bass
import concourse.mybir as mybir
import concourse.tile as tile
from concourse.bass_utils import run_bass_kernel_spmd
from concourse.masks import make_identity

BF16 = ml_dtypes.bfloat16
F32 = np.float32

B, S, E, H = 4, 4096, 1024, 128
NCORES = 8
PROG = [32, 24, 16, 8]                       # program k-tile count per slot
BLOCKS = {0: [7, 5, 2, 0], 1: [6, 4, 3, 1]}  # parity -> owned q-block ids
EC = E // 128                                 # 8 contraction chunks
SB = S // 512                                 # 8 key blocks of 512
QB = 4                                        # q-blocks (slots) per core
QLEN = QB * 512                               # 2048 q rows per core

USE_F32R_DENOM = True                         # pacc in f32r for 1cyc/row L-matmuls

_CACHE = {}


def _build_program():
    dt = mybir.dt
    nc = bacc.Bacc("TRN2", target_bir_lowering=False, debug=False, num_devices=NCORES)

    embT_d = nc.dram_tensor("embT", [E, S], dt.bfloat16, kind="ExternalInput")
    embTq_d = nc.dram_tensor("embTq", [E, QLEN], dt.bfloat16, kind="ExternalInput")
    wq_d = nc.dram_tensor("wq", [E, H], dt.bfloat16, kind="ExternalInput")
    wk_d = nc.dram_tensor("wk", [E, H], dt.bfloat16, kind="ExternalInput")
    wv_d = nc.dram_tensor("wv", [E, H], dt.bfloat16, kind="ExternalInput")
    bq_d = nc.dram_tensor("bq", [H, 1], dt.float32, kind="ExternalInput")
    bk_d = nc.dram_tensor("bk", [H, 1], dt.float32, kind="ExternalInput")
    bv_d = nc.dram_tensor("bv", [H, 1], dt.float32, kind="ExternalInput")
    mask_d = nc.dram_tensor("maskblk", [QB, 128, 8, 512], dt.bfloat16, kind="ExternalInput")
    out_d = nc.dram_tensor("out", [QLEN, H], dt.float32, kind="ExternalOutput")

    ident_f = mybir.ActivationFunctionType.Identity
    exp_f = mybir.ActivationFunctionType.Exp
    dn_dt = dt.float32r if USE_F32R_DENOM else dt.float32

    with tile.TileContext(nc) as tc:
        with tc.tile_pool(name="singles", bufs=1) as singles:
            # ---- constants: weights/biases first on the SP queue (startup path) ----
            w_sb = {}
            b_sb = {}
            for name, wd, bd in (("k", wk_d, bk_d), ("v", wv_d, bv_d), ("q", wq_d, bq_d)):
                w = singles.tile([128, EC, H], dt.bfloat16, tag=f"w{name}", name=f"w{name}")
                nc.sync.dma_start(out=w[:, :, :], in_=wd.ap().rearrange("(c p) h -> p c h", p=128))
                w_sb[name] = w
                b = singles.tile([H, 1], dt.float32, tag=f"b{name}", name=f"b{name}")
                nc.sync.dma_start(out=b[:, :], in_=bd.ap())
                b_sb[name] = b
            identb = singles.tile([128, 128], dt.bfloat16, tag="identb")
            make_identity(nc, identb[:, :])
            identf = singles.tile([128, 128], dt.float32, tag="identf")
            make_identity(nc, identf[:, :])
            ones_f32 = singles.tile([128, 1], dt.float32, tag="ones_f32")
            nc.vector.memset(ones_f32[:, :], 1.0)
            if USE_F32R_DENOM:
                ones_col = singles.tile([128, 1], dn_dt, tag="ones_col")
                nc.vector.tensor_copy(ones_col[:, :], ones_f32[:, :])
            else:
                ones_col = ones_f32
            ones_row = singles.tile([1, 128], dt.float32, tag="ones_row")
            nc.vector.memset(ones_row[:, :], 1.0)

            # per-block tensors, split for fine-grained dependencies
            qTs = [singles.tile([128, 512], dt.bfloat16, tag=f"qT{i}", name=f"qT{i}") for i in range(QB)]
            kTs = [singles.tile([128, 512], dt.bfloat16, tag=f"kT{i}", name=f"kT{i}") for i in range(SB)]
            vTs = [singles.tile([128, 512], dt.bfloat16, tag=f"vT{i}", name=f"vT{i}") for i in range(SB)]
            vts = [singles.tile([128, 128], dt.bfloat16, tag=f"v{i}", name=f"v{i}") for i in range(S // 128)]

            def kv_block(sb, etp, pkp, pvp, ptrvp):
                et = etp.tile([128, EC, 512], dt.bfloat16, tag="et", name=f"et{sb}")
                for c2 in range(EC // 2):
                    eng = nc.sync if (c2 % 2 == 0) else nc.scalar
                    eng.dma_start(
                        out=et[:, 2 * c2:2 * c2 + 2, :],
                        in_=embT_d.ap()
                        .rearrange("(cc c p) s -> p cc c s", c=2, p=128)[:, c2, :, 512 * sb:512 * (sb + 1)],
                    )
                psk = pkp.tile([128, 512], dt.float32, tag="psk", name=f"psk{sb}")
                psv = pvp.tile([128, 512], dt.float32, tag="psv", name=f"psv{sb}")
                for c in range(EC):
                    nc.tensor.matmul(psk[:, :], lhsT=w_sb["k"][:, c, :], rhs=et[:, c, :],
                                     start=(c == 0), stop=(c == EC - 1))
                    nc.tensor.matmul(psv[:, :], lhsT=w_sb["v"][:, c, :], rhs=et[:, c, :],
                                     start=(c == 0), stop=(c == EC - 1))
                nc.scalar.activation(kTs[sb][:, :], psk[:, :], ident_f, bias=b_sb["k"][:, :])
                nc.scalar.activation(vTs[sb][:, :], psv[:, :], ident_f, bias=b_sb["v"][:, :])
                for u in range(4):
                    st = 4 * sb + u
                    tp = ptrvp.tile([128, 128], dt.bfloat16, tag="tp", name=f"tp{st}")
                    nc.tensor.transpose(tp[:, :], vTs[sb][:, 128 * u:128 * (u + 1)], identb[:, :])
                    nc.any.tensor_copy(vts[st][:, :], tp[:, :])

            with tc.tile_pool(name="et", bufs=4) as etp, \
                 tc.tile_pool(name="pk", bufs=2, space="PSUM") as pkp, \
                 tc.tile_pool(name="pv", bufs=2, space="PSUM") as pvp, \
                 tc.tile_pool(name="ptrv", bufs=2, space="PSUM") as ptrvp:
                kv_block(0, etp, pkp, pvp, ptrvp)
                # Q^T projection (attention slot 0 needs qT[0] + kT[0] first)
                for qb2 in range(QB // 2):
                    etq = etp.tile([128, EC, 1024], dt.bfloat16, tag="etq", name=f"etq{qb2}")
                    for c in range(EC):
                        nc.scalar.dma_start(
                            out=etq[:, c, :],
                            in_=embTq_d.ap()[128 * c:128 * (c + 1), 1024 * qb2:1024 * (qb2 + 1)],
                        )
                    for h2 in (0, 1):
                        qb = 2 * qb2 + h2
                        psq = pkp.tile([128, 512], dt.float32, tag="psk", name=f"psq{qb}")
                        for c in range(EC):
                            nc.tensor.matmul(psq[:, :], lhsT=w_sb["q"][:, c, :],
                                             rhs=etq[:, c, 512 * h2:512 * (h2 + 1)],
                                             start=(c == 0), stop=(c == EC - 1))
                        nc.scalar.activation(qTs[qb][:, :], psq[:, :], ident_f, bias=b_sb["q"][:, :])
                for sb in range(1, SB):
                    kv_block(sb, etp, pkp, pvp, ptrvp)

            # ---- attention ----
            with tc.tile_pool(name="ps", bufs=2, space="PSUM") as psp, \
                 tc.tile_pool(name="po", bufs=2, space="PSUM") as pop, \
                 tc.tile_pool(name="pl", bufs=1, space="PSUM") as plp, \
                 tc.tile_pool(name="ptro", bufs=1, space="PSUM") as ptrop, \
                 tc.tile_pool(name="ptbuf", bufs=3) as ptp, \
                 tc.tile_pool(name="mask", bufs=2) as mkp, \
                 tc.tile_pool(name="pacc", bufs=2) as paccp, \
                 tc.tile_pool(name="ep", bufs=2) as epp:
                for s in range(QB):
                    Wp = PROG[s]
                    mt = mkp.tile([128, 8, 512], dt.bfloat16, tag="mt", name=f"mt{s}")
                    nc.sync.dma_start(out=mt[:, :, :], in_=mask_d.ap()[s])
                    po = pop.tile([128, 512], dt.float32, tag="po", name=f"po{s}")
                    pacc_a = paccp.tile([128, 512], dn_dt, tag="pacc_a", name=f"pacc_a{s}")
                    pacc_b = paccp.tile([128, 512], dn_dt, tag="pacc_b", name=f"pacc_b{s}")
                    for p in range(Wp // 2):
                        ps = psp.tile([128, 2, 512], dt.float32, tag="ps", name=f"ps{s}_{p}")
                        for h2 in (0, 1):
                            t = 2 * p + h2
                            nc.tensor.matmul(ps[:, h2, :],
                                             lhsT=kTs[t // 4][:, 128 * (t % 4):128 * (t % 4 + 1)],
                                             rhs=qTs[s][:, :], start=True, stop=True)
                        pt = ptp.tile([128, 2, 512], dt.bfloat16, tag="pt", name=f"pt{s}_{p}")
                        nc.scalar.activation(pt[:, :, :], ps[:, :, :], exp_f)
                        for h2 in (0, 1):
                            t = 2 * p + h2
                            if t >= Wp - 8:
                                nc.vector.tensor_mul(pt[:, h2, :], pt[:, h2, :],
                                                     mt[:, t - (Wp - 8), :])
                        # softmax denominator partials: even k-tiles on DVE, odd on GPSIMD
                        if p == 0:
                            nc.vector.tensor_copy(pacc_a[:, :], pt[:, 0, :])
                            nc.gpsimd.tensor_copy(pacc_b[:, :], pt[:, 1, :])
                        else:
                            nc.vector.tensor_add(pacc_a[:, :], pacc_a[:, :], pt[:, 0, :])
                            nc.gpsimd.tensor_add(pacc_b[:, :], pacc_b[:, :], pt[:, 1, :])
                        for h2 in (0, 1):
                            t = 2 * p + h2
                            nc.tensor.matmul(po[:, :], lhsT=vts[t][:, :],
                                             rhs=pt[:, h2, :], start=(t == 0), stop=(t == Wp - 1))
                    # epilogue: L = colsum(pacc_a + pacc_b); out = (po / L).T
                    pl = plp.tile([1, 512], dt.float32, tag="pl", name=f"pl{s}")
                    nc.tensor.matmul(pl[:, :], lhsT=ones_col[:, :], rhs=pacc_a[:, :],
                                     start=True, stop=False)
                    nc.tensor.matmul(pl[:, :], lhsT=ones_col[:, :], rhs=pacc_b[:, :],
                                     start=False, stop=True)
                    recip = epp.tile([1, 512], dt.float32, tag="recip", name=f"recip{s}")
                    nc.vector.reciprocal(recip[:, :], pl[:, :])
                    pb = plp.tile([128, 512], dt.float32, tag="pl", name=f"pb{s}")
                    nc.tensor.matmul(pb[:, :], lhsT=ones_row[:, :], rhs=recip[:, :],
                                     start=True, stop=True)
                    rb_sb = epp.tile([128, 512], dt.float32, tag="rb_sb", name=f"rb{s}")
                    nc.vector.tensor_copy(rb_sb[:, :], pb[:, :])
                    onrm = epp.tile([128, 512], dt.float32, tag="onrm", name=f"onrm{s}")
                    nc.vector.tensor_mul(onrm[:, :], po[:, :], rb_sb[:, :])
                    oc = epp.tile([128, 4, 128], dt.float32, tag="oc", name=f"oc{s}")
                    for u in range(4):
                        tp2 = ptrop.tile([128, 128], dt.float32, tag="tp2", name=f"tp2_{s}_{u}")
                        nc.tensor.transpose(tp2[:, :], onrm[:, 128 * u:128 * (u + 1)],
                                            identf[:, :])
                        nc.vector.tensor_copy(oc[:, u, :], tp2[:, :])
                    nc.sync.dma_start(
                        out=out_d.ap()[512 * s:512 * (s + 1), :].rearrange("(u p) h -> p u h", p=128),
                        in_=oc[:, :, :],
                    )

    nc.compile()
    return nc


def _build_maskblk(parity):
    m = np.zeros((QB, 128, 8, 512), np.float32)
    kk = np.arange(128)[:, None]
    qq = np.arange(512)[None, :]
    for s, j in enumerate(BLOCKS[parity]):
        Wp, Wa = PROG[s], 4 * (j + 1)
        for i in range(8):
            d = (Wp - 8 + i) - (Wa - 4)
            m[s, :, i, :] = ((qq - 128 * d) >= kk)
    return m.astype(BF16)


def kernel(embds, Wq, bq, Wk, bk, Wv, bv):
    embds = np.asarray(embds, F32)
    Wq = np.asarray(Wq, F32); bq = np.asarray(bq, F32)
    Wk = np.asarray(Wk, F32); bk = np.asarray(bk, F32)
    Wv = np.asarray(Wv, F32); bv = np.asarray(bv, F32)

    if "nc" not in _CACHE:
        _CACHE["nc"] = _build_program()
    nc = _CACHE["nc"]

    scale = F32(1.0 / np.sqrt(H))
    wq_h = (Wq * scale).astype(BF16)
    wk_h = Wk.astype(BF16)
    wv_h = Wv.astype(BF16)
    bq_h = (bq * scale).astype(F32).reshape(H, 1)
    bk_h = bk.astype(F32).reshape(H, 1)
    bv_h = bv.astype(F32).reshape(H, 1)
    masks = {p: _build_maskblk(p) for p in (0, 1)}

    embT = {b: np.ascontiguousarray(embds[b].T).astype(BF16) for b in range(B)}

    in_maps = []
    for c in range(NCORES):
        b, parity = c // 2, c % 2
        et = embT[b]
        etq = np.concatenate([et[:, 512 * j:512 * (j + 1)] for j in BLOCKS[parity]], axis=1)
        in_maps.append({
            "embT": et,
            "embTq": np.ascontiguousarray(etq),
            "wq": wq_h, "wk": wk_h, "wv": wv_h,
            "bq": bq_h, "bk": bk_h, "bv": bv_h,
            "maskblk": masks[parity],
        })

    res = run_bass_kernel_spmd(nc, in_maps, list(range(NCORES)))

    out = np.empty((B, S, H), F32)
    for c in range(NCORES):
        b, parity = c // 2, c % 2
        oc = res.results[c]["out"]
        for s, j in enumerate(BLOCKS[parity]):
            out[b, 512 * j:512 * (j + 1)] = oc[512 * s:512 * (s + 1)]
    return out



# revision 44
# speedup vs baseline: 1.4136x; 1.4136x over previous
"""Causal single-head attention (B=4, S=4096, E=1024, H=128) on 8 TRN2 NeuronCores.

Sharding: 8 cores = 4 batches x 2 sequence shards. Each core owns 4 query
blocks of 512 rows of one batch (parity split {7,5,2,0} / {6,4,3,1}); causal
work is padded to a uniform program of [32,24,16,8] k-tiles per slot so all 8
cores run one SPMD program. Per-core inputs (gathered Q columns + causal
masks) encode which q-blocks a core owns.

Schedule: K/V projection blocks are interleaved INTO the attention slots as
PE filler so the tensor engine never idles while the ACT engine runs exp.
Slots are processed smallest-first [8,16,24,32] so early slots only need the
first key blocks. V is projected directly in [keys, H] layout (lhsT=embT
slice, rhs=Wv) so no PE transposes are needed; K-bias is dropped (softmax is
invariant to per-query constants) and V-bias is folded into the output
epilogue. Softmax denominator: exp pairs accumulated in fp16 on DVE, reduced
across partitions on GPSIMD, reciprocal on DVE. Output is normalized to fp16,
transposed by the DMA XBAR, and written out as fp16 (host upcasts).
"""

import os
import numpy as np
import ml_dtypes

import concourse.bacc as bacc
import concourse.bass_isa as bass_isa
import concourse.mybir as mybir
import concourse.tile as tile
from concourse.bass_utils import run_bass_kernel_spmd

BF16 = ml_dtypes.bfloat16
FP16 = np.float16
FP8 = ml_dtypes.float8_e4m3
F32 = np.float32

B, S, E, H = 4, 4096, 1024, 128
NCORES = 8
PROG = [32, 24, 16, 8]                       # program k-tile count per slot
BLOCKS = {0: [7, 5, 2, 0], 1: [6, 4, 3, 1]}  # parity -> owned q-block ids
# per-parity key-block permutation: position p of the core's embT holds real
# block PI[parity][p]. Chosen so each slot's q-block sits at the fixed
# position POS[s] (so Q-projection reads the et tiles; no separate q gather)
# while every slot's allowed key set remains a prefix of positions.
PI = {0: [0, 1, 2, 3, 5, 4, 7, 6], 1: [1, 0, 3, 2, 4, 5, 6, 7]}
POS = {3: 0, 2: 2, 1: 4, 0: 6}               # slot -> position of its q-block
EC = E // 128                                 # 8 contraction chunks
SB = S // 512                                 # 8 key blocks of 512
QB = 4                                        # q-blocks (slots) per core
QLEN = QB * 512                               # 2048 q rows per core

_CACHE = {}


def _build_program():
    dt = mybir.dt
    nc = bacc.Bacc("TRN2", target_bir_lowering=False, debug=False, num_devices=NCORES)

    embT_d = nc.dram_tensor("embT", [E, S], dt.bfloat16, kind="ExternalInput")
    # weights pre-arranged on host to [128, EC*H] (partition-major chunks)
    wq_d = nc.dram_tensor("wq", [128, EC * H], dt.bfloat16, kind="ExternalInput")
    wk_d = nc.dram_tensor("wk", [128, EC * H], dt.bfloat16, kind="ExternalInput")
    wv_d = nc.dram_tensor("wv", [128, EC * H], dt.bfloat16, kind="ExternalInput")
    bq_d = nc.dram_tensor("bq", [H, 1], dt.float32, kind="ExternalInput")
    bv_d = nc.dram_tensor("bv", [H, 1], dt.float32, kind="ExternalInput")
    mask_d = nc.dram_tensor("maskblk", [QB, 128, 8, 512], dt.float16, kind="ExternalInput")
    # output stays transposed [H, QLEN]; host transposes (part of unshard)
    out_d = nc.dram_tensor("outT", [H, QLEN], dt.float16, kind="ExternalOutput")
    dbg = {}
    if os.environ.get("KDEBUG"):
        dbg["kT0"] = nc.dram_tensor("dbg_kT0", [128, 512], dt.bfloat16, kind="ExternalOutput")
        dbg["vt0"] = nc.dram_tensor("dbg_vt0", [128, 4, 128], dt.float16, kind="ExternalOutput")
        dbg["qT3"] = nc.dram_tensor("dbg_qT3", [128, 512], dt.bfloat16, kind="ExternalOutput")
        dbg["pacc3"] = nc.dram_tensor("dbg_pacc3", [128, 512], dt.float16, kind="ExternalOutput")
        dbg["lall3"] = nc.dram_tensor("dbg_lall3", [128, 512], dt.float32, kind="ExternalOutput")
        dbg["po3"] = nc.dram_tensor("dbg_po3", [128, 512], dt.float16, kind="ExternalOutput")

    ident_f = mybir.ActivationFunctionType.Identity
    exp_f = mybir.ActivationFunctionType.Exp

    with tile.TileContext(nc) as tc:
        with tc.tile_pool(name="singles", bufs=1) as singles, \
             tc.tile_pool(name="et", bufs=4) as etp, \
             tc.tile_pool(name="pk", bufs=1, space="PSUM") as pkp, \
             tc.tile_pool(name="pv", bufs=1, space="PSUM") as pvp, \
             tc.tile_pool(name="ps", bufs=2, space="PSUM") as psp, \
             tc.tile_pool(name="po", bufs=2, space="PSUM") as pop, \
             tc.tile_pool(name="mask", bufs=4) as mkp, \
             tc.tile_pool(name="pt", bufs=3) as ptp, \
             tc.tile_pool(name="pacc", bufs=2) as pacp, \
             tc.tile_pool(name="ep", bufs=2) as epp:

            # ---- constant tiles ----
            w_sb = {}
            for name in ("k", "v", "q"):
                w_sb[name] = singles.tile([128, EC, H], dt.bfloat16, tag=f"w{name}", name=f"w{name}")
            bq_sb = singles.tile([H, 1], dt.float32, tag="bq")
            bv_sb = singles.tile([H, 1], dt.float32, tag="bv")
            kTs = [singles.tile([128, 512], dt.bfloat16, tag=f"kT{i}", name=f"kT{i}") for i in range(SB)]
            vt4 = [singles.tile([128, 4, 128], dt.float16, tag=f"v{i}", name=f"v{i}") for i in range(SB)]
            qTs = [singles.tile([128, 512], dt.bfloat16, tag=f"qT{i}", name=f"qT{i}") for i in range(QB)]

            ets = {}
            mts = {}

            # ---- PE pre-ramp: dummy matmuls during the DMA-bound startup so
            # the tensor engine reaches full p-state before real work arrives
            dums = singles.tile([128, 512], dt.bfloat16, tag="dums")
            psd = psp.tile([128, 2, 512], dt.float32, tag="ps", name="psd")
            nc.vector.memset(dums[:, :], 0.0)
            for i in range(14):
                nc.tensor.matmul(psd[:, i % 2, :], lhsT=dums[:, 0:128], rhs=dums[:, :],
                                 start=True, stop=True)

            # ---- startup DMA burst (need-ordered; sync + scalar alternate) ----
            nc.sync.dma_start(out=w_sb["k"][:, 0, :], in_=wk_d.ap()[:, 0:H])
            et0 = etp.tile([128, EC, 512], dt.bfloat16, tag="et", name="et0")
            ets[0] = et0
            nc.scalar.dma_start(
                out=et0[:, 0:2, :],
                in_=embT_d.ap().rearrange("(c p) s -> p c s", p=128)[:, 0:2, 0:512])
            nc.sync.dma_start(out=w_sb["k"][:, 1:EC, :],
                              in_=wk_d.ap()[:, H:EC * H].rearrange("p (c h) -> p c h", h=H))
            nc.scalar.dma_start(
                out=et0[:, 2:4, :],
                in_=embT_d.ap().rearrange("(c p) s -> p c s", p=128)[:, 2:4, 0:512])
            nc.sync.dma_start(out=w_sb["v"][:, :, :],
                              in_=wv_d.ap().rearrange("p (c h) -> p c h", h=H))
            nc.scalar.dma_start(
                out=et0[:, 4:6, :],
                in_=embT_d.ap().rearrange("(c p) s -> p c s", p=128)[:, 4:6, 0:512])
            nc.sync.dma_start(out=bq_sb[:, :], in_=bq_d.ap())
            nc.scalar.dma_start(
                out=et0[:, 6:8, :],
                in_=embT_d.ap().rearrange("(c p) s -> p c s", p=128)[:, 6:8, 0:512])
            nc.sync.dma_start(out=bv_sb[:, :], in_=bv_d.ap())
            nc.scalar.dma_start(out=w_sb["q"][:, :, :],
                                in_=wq_d.ap().rearrange("p (c h) -> p c h", h=H))
            # remaining input DMAs in strict need-order (the DMA pipe is the
            # prologue bottleneck; transfers execute in HWDGE-issue order)

            def dma_et(b, eng):
                t = etp.tile([128, EC, 512], dt.bfloat16, tag="et", name=f"et{b}")
                ets[b] = t
                eng.dma_start(
                    out=t[:, :, :],
                    in_=embT_d.ap().rearrange("(c p) s -> p c s", p=128)[:, :, 512 * b:512 * (b + 1)])

            def dma_mask(s, eng):
                t = mkp.tile([128, 8, 512], dt.float16, tag="mt", name=f"mt{s}")
                mts[s] = t
                eng.dma_start(out=t[:, :, :], in_=mask_d.ap()[s])

            # ---- kv block emitters: 16 PE units (8 K-chunks, 8 V-chunks) ----
            def kv_units(b):
                units = []
                et = ets[b]
                psk = pkp.tile([128, 512], dt.float32, tag="psk", name=f"psk{b}")
                psv = pvp.tile([128, 4, 128], dt.float32, tag="psv", name=f"psv{b}")

                def k_chunk(c):
                    def emit():
                        nc.tensor.matmul(psk[:, :], lhsT=w_sb["k"][:, c, :], rhs=et[:, c, :],
                                         start=(c == 0), stop=(c == EC - 1))
                        if c == EC - 1:
                            nc.vector.tensor_copy(kTs[b][:, :], psk[:, :])
                    return emit

                def v_unit(u):
                    # one full accumulation group per bank region; groups in the
                    # same PSUM bank must not interleave (codegen breaks)
                    def emit():
                        for c in range(EC):
                            nc.tensor.matmul(psv[:, u, :],
                                             lhsT=et[:, c, 128 * u:128 * (u + 1)],
                                             rhs=w_sb["v"][:, c, :],
                                             start=(c == 0), stop=(c == EC - 1))
                        if u == 3:
                            nc.vector.tensor_copy(vt4[b][:, :, :], psv[:, :, :])
                    return emit

                for c in range(EC):
                    units.append(k_chunk(c))
                for u in range(4):
                    units.append(v_unit(u))
                return units

            def qproj(s):
                etq = ets[POS[s]]
                psq = pop.tile([128, 512], dt.float32, tag="po", name=f"psq{s}")
                for c in range(EC):
                    nc.tensor.matmul(psq[:, :], lhsT=w_sb["q"][:, c, :], rhs=etq[:, c, :],
                                     start=(c == 0), stop=(c == EC - 1))
                nc.scalar.activation(qTs[s][:, :], psq[:, :], ident_f, bias=bq_sb[:, :])

            # ---- attention slot with PE filler consumption ----
            def slot(s, fillq):
                Wp = PROG[s]
                P = Wp // 2
                mt = mts[s]
                po = pop.tile([128, 512], dt.float32, tag="po", name=f"po{s}")
                pacc = pacp.tile([128, 2, 512], dt.float16, tag="pacc", name=f"pacc{s}")
                pss, pts = {}, {}
                q0 = len(fillq)

                def scores(p):
                    ps = psp.tile([128, 2, 512], dt.float32, tag="ps", name=f"ps{s}_{p}")
                    pss[p] = ps
                    for h2 in (0, 1):
                        t = 2 * p + h2
                        nc.tensor.matmul(ps[:, h2, :],
                                         lhsT=kTs[t // 4][:, 128 * (t % 4):128 * (t % 4 + 1)],
                                         rhs=qTs[s][:, :], start=True, stop=True)

                def av(p):
                    pt = pts[p]
                    for h2 in (0, 1):
                        t = 2 * p + h2
                        nc.tensor.matmul(po[:, :], lhsT=vt4[t // 4][:, t % 4, :],
                                         rhs=pt[:, h2, :],
                                         start=(t == 0), stop=(t == Wp - 1))

                def exp_mask_acc(p):
                    ps = pss.pop(p)
                    pt = ptp.tile([128, 2, 512], dt.float16, tag="pt", name=f"pt{s}_{p}")
                    pts[p] = pt
                    nc.scalar.activation(pt[:, :, :], ps[:, :, :], exp_f)
                    for h2 in (0, 1):
                        t = 2 * p + h2
                        if t >= Wp - 8:
                            nc.vector.tensor_mul(pt[:, h2, :], pt[:, h2, :],
                                                 mt[:, t - (Wp - 8), :])
                    if p == 0:
                        nc.vector.tensor_copy(pacc[:, :, :], pt[:, :, :])
                    else:
                        nc.vector.tensor_add(pacc[:, :, :], pacc[:, :, :], pt[:, :, :])

                for p in range(P):
                    scores(p)
                    if p >= 1:
                        av(p - 1)
                        pts.pop(p - 1)
                    exp_mask_acc(p)
                    # evenly paced filler consumption across the slot
                    target = -(-q0 * (p + 1) // P)
                    while q0 - len(fillq) < target and fillq:
                        fillq.pop(0)()
                av(P - 1)
                pts.pop(P - 1)

                # epilogue, pipelined over column halves
                pacc1 = epp.tile([128, 512], dt.float16, tag="pacc1", name=f"pacc1{s}")
                lall = epp.tile([128, 512], dt.float32, tag="lall", name=f"lall{s}")
                rb = epp.tile([128, 512], dt.float32, tag="rb", name=f"rb{s}")
                tmpo = epp.tile([128, 512], dt.float16, tag="tmpo", name=f"tmpo{s}")
                onrm = epp.tile([128, 512], dt.float16, tag="onrm", name=f"onrm{s}")
                chunks = ((slice(0, 128), slice(128, 256), slice(256, 384), slice(384, 512))
                          if s == 0 else (slice(0, 256), slice(256, 512)))
                for hh in chunks:
                    nc.vector.tensor_add(pacc1[:, hh], pacc[:, 0, hh], pacc[:, 1, hh])
                    nc.gpsimd.partition_all_reduce(lall[:, hh], pacc1[:, hh], 128,
                                                   bass_isa.ReduceOp.add)
                    nc.vector.reciprocal(rb[:, hh], lall[:, hh])
                    nc.vector.tensor_mul(tmpo[:, hh], po[:, hh], rb[:, hh])
                    nc.vector.tensor_scalar_add(onrm[:, hh], tmpo[:, hh], bv_sb[:, :])
                    # out stays [H, q] in DRAM; host transposes. Last slot uses
                    # the (by then idle) SP HWDGE queue for lower latency.
                    eng = nc.sync if s == 0 else nc.gpsimd
                    eng.dma_start(out=out_d.ap()[:, 512 * s + hh.start:512 * s + hh.stop],
                                  in_=onrm[:, hh])
                if dbg and s == 3:
                    nc.sync.dma_start(out=dbg["kT0"].ap(), in_=kTs[0][:, :])
                    nc.sync.dma_start(out=dbg["vt0"].ap(), in_=vt4[0][:, :, :])
                    nc.sync.dma_start(out=dbg["qT3"].ap(), in_=qTs[3][:, :])
                    nc.sync.dma_start(out=dbg["pacc3"].ap(), in_=pacc1[:, :])
                    nc.sync.dma_start(out=dbg["lall3"].ap(), in_=lall[:, :])
                    nc.sync.dma_start(out=dbg["po3"].ap(), in_=tmpo[:, :])

            # ================= phase schedule =================
            dma_et(1, nc.sync)
            dma_mask(3, nc.sync)
            for u in kv_units(0):
                u()
            qproj(3)
            dma_et(2, nc.sync)
            for u in kv_units(1):
                u()

            fillq = []
            slot(3, fillq)

            # kv2's et lands mid-slot3; run it directly after
            for u in kv_units(2):
                u()
            qproj(2)
            dma_et(3, nc.sync)
            dma_mask(2, nc.sync)
            fillq += kv_units(3)
            dma_et(4, nc.sync)
            fillq += kv_units(4)
            dma_et(5, nc.sync)
            dma_mask(1, nc.sync)
            slot(2, fillq)

            while fillq:
                fillq.pop(0)()
            qproj(1)
            fillq += kv_units(5)
            dma_et(6, nc.sync)
            fillq += kv_units(6)
            dma_et(7, nc.sync)
            dma_mask(0, nc.sync)
            slot(1, fillq)

            while fillq:
                fillq.pop(0)()
            qproj(0)
            fillq += kv_units(7)
            slot(0, fillq)

    nc.compile()
    return nc


def _build_maskblk(parity):
    m = np.zeros((QB, 128, 8, 512), np.float32)
    kk = np.arange(128)[:, None]
    qq = np.arange(512)[None, :]
    pi = PI[parity]
    for s, j in enumerate(BLOCKS[parity]):
        Wp = PROG[s]
        for i in range(8):
            t = Wp - 8 + i
            d = 4 * (pi[t // 4] - j) + t % 4
            m[s, :, i, :] = ((qq - 128 * d) >= kk)
    return m.astype(FP16)


def _rearrange_w(w):
    # [E, H] -> [128, EC*H] with chunk-major free dim
    return np.ascontiguousarray(
        w.reshape(EC, 128, H).transpose(1, 0, 2).reshape(128, EC * H)).astype(BF16)


def kernel(embds, Wq, bq, Wk, bk, Wv, bv):
    embds = np.asarray(embds, F32)
    Wq = np.asarray(Wq, F32); bq = np.asarray(bq, F32)
    Wk = np.asarray(Wk, F32)
    Wv = np.asarray(Wv, F32); bv = np.asarray(bv, F32)

    if "nc" not in _CACHE:
        _CACHE["nc"] = _build_program()
    nc = _CACHE["nc"]

    scale = F32(1.0 / np.sqrt(H))
    wq_h = _rearrange_w(Wq * scale)
    wk_h = _rearrange_w(Wk)
    wv_h = _rearrange_w(Wv)
    bq_h = (bq * scale).astype(F32).reshape(H, 1)
    bv_h = bv.astype(F32).reshape(H, 1)
    masks = {p: _build_maskblk(p) for p in (0, 1)}

    embT = {b: np.ascontiguousarray(embds[b].T).astype(BF16) for b in range(B)}
    # per-parity key-block permutation of the embedding columns (see PI)
    embTp = {}
    for b in range(B):
        for parity in (0, 1):
            et = embT[b]
            embTp[(b, parity)] = np.ascontiguousarray(
                np.concatenate([et[:, 512 * r:512 * (r + 1)] for r in PI[parity]], axis=1))

    in_maps = []
    for c in range(NCORES):
        b, parity = c // 2, c % 2
        in_maps.append({
            "embT": embTp[(b, parity)],
            "wq": wq_h, "wk": wk_h, "wv": wv_h,
            "bq": bq_h, "bv": bv_h,
            "maskblk": masks[parity],
        })

    res = run_bass_kernel_spmd(nc, in_maps, list(range(NCORES)))
    if os.environ.get("KDEBUG"):
        _CACHE["dbg"] = res.results[0]

    out = np.empty((B, S, H), F32)
    for c in range(NCORES):
        b, parity = c // 2, c % 2
        oc = res.results[c]["outT"].astype(F32).T
        for s, j in enumerate(BLOCKS[parity]):
            out[b, 512 * j:512 * (j + 1)] = oc[512 * s:512 * (s + 1)]
    return out


# revision 45
# speedup vs baseline: 1.4297x; 1.0114x over previous
"""Causal single-head attention (B=4, S=4096, E=1024, H=128) on 8 TRN2 NeuronCores.

Sharding: 8 cores = 4 batches x 2 sequence shards. Each core owns 4 query
blocks of 512 rows of one batch (parity split {7,5,2,0} / {6,4,3,1}); causal
work is padded to a uniform program of [32,24,16,8] k-tiles per slot so all 8
cores run one SPMD program. Per-core inputs (gathered Q columns + causal
masks) encode which q-blocks a core owns.

Schedule: K/V projection blocks are interleaved INTO the attention slots as
PE filler so the tensor engine never idles while the ACT engine runs exp.
Slots are processed smallest-first [8,16,24,32] so early slots only need the
first key blocks. V is projected directly in [keys, H] layout (lhsT=embT
slice, rhs=Wv) so no PE transposes are needed; K-bias is dropped (softmax is
invariant to per-query constants) and V-bias is folded into the output
epilogue. Softmax denominator: exp pairs accumulated in fp16 on DVE, reduced
across partitions on GPSIMD, reciprocal on DVE. Output is normalized to fp16,
transposed by the DMA XBAR, and written out as fp16 (host upcasts).
"""

import os
import numpy as np
import ml_dtypes

import concourse.bacc as bacc
import concourse.bass_isa as bass_isa
import concourse.mybir as mybir
import concourse.tile as tile
from concourse.bass_utils import run_bass_kernel_spmd

BF16 = ml_dtypes.bfloat16
FP16 = np.float16
FP8 = ml_dtypes.float8_e4m3
F32 = np.float32

B, S, E, H = 4, 4096, 1024, 128
NCORES = 8
PROG = [32, 24, 16, 8]                       # program k-tile count per slot
BLOCKS = {0: [7, 5, 2, 0], 1: [6, 4, 3, 1]}  # parity -> owned q-block ids
# per-parity key-block permutation: position p of the core's embT holds real
# block PI[parity][p]. Chosen so each slot's q-block sits at the fixed
# position POS[s] (so Q-projection reads the et tiles; no separate q gather)
# while every slot's allowed key set remains a prefix of positions.
PI = {0: [0, 1, 2, 3, 5, 4, 7, 6], 1: [1, 0, 3, 2, 4, 5, 6, 7]}
POS = {3: 0, 2: 2, 1: 4, 0: 6}               # slot -> position of its q-block
EC = E // 128                                 # 8 contraction chunks
SB = S // 512                                 # 8 key blocks of 512
QB = 4                                        # q-blocks (slots) per core
QLEN = QB * 512                               # 2048 q rows per core

_CACHE = {}


def _build_program():
    dt = mybir.dt
    nc = bacc.Bacc("TRN2", target_bir_lowering=False, debug=False, num_devices=NCORES)

    embT_d = nc.dram_tensor("embT", [E, S], dt.bfloat16, kind="ExternalInput")
    # weights pre-arranged on host to [128, EC*H] (partition-major chunks)
    wq_d = nc.dram_tensor("wq", [128, EC * H], dt.bfloat16, kind="ExternalInput")
    wk_d = nc.dram_tensor("wk", [128, EC * H], dt.bfloat16, kind="ExternalInput")
    wv_d = nc.dram_tensor("wv", [128, EC * H], dt.bfloat16, kind="ExternalInput")
    bq_d = nc.dram_tensor("bq", [H, 1], dt.float32, kind="ExternalInput")
    bv_d = nc.dram_tensor("bv", [H, 1], dt.float32, kind="ExternalInput")
    mask_d = nc.dram_tensor("maskblk", [QB, 128, 8, 512], dt.float16, kind="ExternalInput")
    # output stays transposed [H, QLEN]; host transposes (part of unshard)
    out_d = nc.dram_tensor("outT", [H, QLEN], dt.float16, kind="ExternalOutput")
    dbg = {}
    if os.environ.get("KDEBUG"):
        dbg["kT0"] = nc.dram_tensor("dbg_kT0", [128, 512], dt.bfloat16, kind="ExternalOutput")
        dbg["vt0"] = nc.dram_tensor("dbg_vt0", [128, 4, 128], dt.float16, kind="ExternalOutput")
        dbg["qT3"] = nc.dram_tensor("dbg_qT3", [128, 512], dt.bfloat16, kind="ExternalOutput")
        dbg["pacc3"] = nc.dram_tensor("dbg_pacc3", [128, 512], dt.float16, kind="ExternalOutput")
        dbg["lall3"] = nc.dram_tensor("dbg_lall3", [128, 512], dt.float32, kind="ExternalOutput")
        dbg["po3"] = nc.dram_tensor("dbg_po3", [128, 512], dt.float16, kind="ExternalOutput")

    ident_f = mybir.ActivationFunctionType.Identity
    exp_f = mybir.ActivationFunctionType.Exp

    with tile.TileContext(nc) as tc:
        with tc.tile_pool(name="singles", bufs=1) as singles, \
             tc.tile_pool(name="et", bufs=4) as etp, \
             tc.tile_pool(name="pk", bufs=1, space="PSUM") as pkp, \
             tc.tile_pool(name="pv", bufs=1, space="PSUM") as pvp, \
             tc.tile_pool(name="ps", bufs=2, space="PSUM") as psp, \
             tc.tile_pool(name="po", bufs=2, space="PSUM") as pop, \
             tc.tile_pool(name="mask", bufs=4) as mkp, \
             tc.tile_pool(name="pt", bufs=3) as ptp, \
             tc.tile_pool(name="pacc", bufs=2) as pacp, \
             tc.tile_pool(name="ep", bufs=2) as epp:

            # ---- constant tiles ----
            w_sb = {}
            for name in ("k", "v", "q"):
                w_sb[name] = singles.tile([128, EC, H], dt.bfloat16, tag=f"w{name}", name=f"w{name}")
            bq_sb = singles.tile([H, 1], dt.float32, tag="bq")
            bv_sb = singles.tile([H, 1], dt.float32, tag="bv")
            kTs = [singles.tile([128, 512], dt.bfloat16, tag=f"kT{i}", name=f"kT{i}") for i in range(SB)]
            vt4 = [singles.tile([128, 4, 128], dt.float16, tag=f"v{i}", name=f"v{i}") for i in range(SB)]
            qTs = [singles.tile([128, 512], dt.bfloat16, tag=f"qT{i}", name=f"qT{i}") for i in range(QB)]

            ets = {}
            mts = {}

            # ---- PE pre-ramp: dummy matmuls during the DMA-bound startup so
            # the tensor engine reaches full p-state before real work arrives
            dums = singles.tile([128, 512], dt.bfloat16, tag="dums")
            psd = psp.tile([128, 2, 512], dt.float32, tag="ps", name="psd")
            nc.vector.memset(dums[:, :], 0.0)
            for i in range(14):
                nc.tensor.matmul(psd[:, i % 2, :], lhsT=dums[:, 0:128], rhs=dums[:, :],
                                 start=True, stop=True)

            # ---- startup DMA burst (need-ordered; sync + scalar alternate) ----
            nc.sync.dma_start(out=w_sb["k"][:, 0, :], in_=wk_d.ap()[:, 0:H])
            et0 = etp.tile([128, EC, 512], dt.bfloat16, tag="et", name="et0")
            ets[0] = et0
            nc.scalar.dma_start(
                out=et0[:, 0:2, :],
                in_=embT_d.ap().rearrange("(c p) s -> p c s", p=128)[:, 0:2, 0:512])
            nc.sync.dma_start(out=w_sb["k"][:, 1:EC, :],
                              in_=wk_d.ap()[:, H:EC * H].rearrange("p (c h) -> p c h", h=H))
            nc.scalar.dma_start(
                out=et0[:, 2:4, :],
                in_=embT_d.ap().rearrange("(c p) s -> p c s", p=128)[:, 2:4, 0:512])
            nc.sync.dma_start(out=w_sb["v"][:, :, :],
                              in_=wv_d.ap().rearrange("p (c h) -> p c h", h=H))
            nc.scalar.dma_start(
                out=et0[:, 4:6, :],
                in_=embT_d.ap().rearrange("(c p) s -> p c s", p=128)[:, 4:6, 0:512])
            nc.sync.dma_start(out=bq_sb[:, :], in_=bq_d.ap())
            nc.scalar.dma_start(
                out=et0[:, 6:8, :],
                in_=embT_d.ap().rearrange("(c p) s -> p c s", p=128)[:, 6:8, 0:512])
            nc.sync.dma_start(out=bv_sb[:, :], in_=bv_d.ap())
            nc.scalar.dma_start(out=w_sb["q"][:, :, :],
                                in_=wq_d.ap().rearrange("p (c h) -> p c h", h=H))
            # remaining input DMAs in strict need-order (the DMA pipe is the
            # prologue bottleneck; transfers execute in HWDGE-issue order)

            def dma_et(b, eng):
                t = etp.tile([128, EC, 512], dt.bfloat16, tag="et", name=f"et{b}")
                ets[b] = t
                eng.dma_start(
                    out=t[:, :, :],
                    in_=embT_d.ap().rearrange("(c p) s -> p c s", p=128)[:, :, 512 * b:512 * (b + 1)])

            def dma_mask(s, eng):
                t = mkp.tile([128, 8, 512], dt.float16, tag="mt", name=f"mt{s}")
                mts[s] = t
                eng.dma_start(out=t[:, :, :], in_=mask_d.ap()[s])

            # ---- kv block emitters: 16 PE units (8 K-chunks, 8 V-chunks) ----
            def kv_units(b):
                units = []
                et = ets[b]
                psk = pkp.tile([128, 512], dt.float32, tag="psk", name=f"psk{b}")
                psv = pvp.tile([128, 4, 128], dt.float32, tag="psv", name=f"psv{b}")

                def k_chunk(c):
                    def emit():
                        nc.tensor.matmul(psk[:, :], lhsT=w_sb["k"][:, c, :], rhs=et[:, c, :],
                                         start=(c == 0), stop=(c == EC - 1))
                        if c == EC - 1:
                            nc.vector.tensor_copy(kTs[b][:, :], psk[:, :])
                    return emit

                def v_unit(u):
                    # one full accumulation group per bank region; groups in the
                    # same PSUM bank must not interleave (codegen breaks)
                    def emit():
                        for c in range(EC):
                            nc.tensor.matmul(psv[:, u, :],
                                             lhsT=et[:, c, 128 * u:128 * (u + 1)],
                                             rhs=w_sb["v"][:, c, :],
                                             start=(c == 0), stop=(c == EC - 1))
                        if u == 3:
                            nc.vector.tensor_copy(vt4[b][:, :, :], psv[:, :, :])
                    return emit

                for c in range(EC):
                    units.append(k_chunk(c))
                for u in range(4):
                    units.append(v_unit(u))
                return units

            def qproj(s):
                etq = ets[POS[s]]
                psq = pop.tile([128, 512], dt.float32, tag="po", name=f"psq{s}")
                for c in range(EC):
                    nc.tensor.matmul(psq[:, :], lhsT=w_sb["q"][:, c, :], rhs=etq[:, c, :],
                                     start=(c == 0), stop=(c == EC - 1))
                nc.scalar.activation(qTs[s][:, :], psq[:, :], ident_f, bias=bq_sb[:, :])

            # ---- attention slot with PE filler consumption ----
            def slot(s, fillq):
                Wp = PROG[s]
                P = Wp // 2
                mt = mts[s]
                po = pop.tile([128, 512], dt.float32, tag="po", name=f"po{s}")
                pacc = pacp.tile([128, 2, 512], dt.float16, tag="pacc", name=f"pacc{s}")
                pss, pts = {}, {}
                q0 = len(fillq)

                def scores(p):
                    ps = psp.tile([128, 2, 512], dt.float32, tag="ps", name=f"ps{s}_{p}")
                    pss[p] = ps
                    for h2 in (0, 1):
                        t = 2 * p + h2
                        nc.tensor.matmul(ps[:, h2, :],
                                         lhsT=kTs[t // 4][:, 128 * (t % 4):128 * (t % 4 + 1)],
                                         rhs=qTs[s][:, :], start=True, stop=True)

                def av(p):
                    pt = pts[p]
                    for h2 in (0, 1):
                        t = 2 * p + h2
                        nc.tensor.matmul(po[:, :], lhsT=vt4[t // 4][:, t % 4, :],
                                         rhs=pt[:, h2, :],
                                         start=(t == 0), stop=(t == Wp - 1))

                def exp_mask_acc(p):
                    ps = pss.pop(p)
                    pt = ptp.tile([128, 2, 512], dt.float16, tag="pt", name=f"pt{s}_{p}")
                    pts[p] = pt
                    nc.scalar.activation(pt[:, :, :], ps[:, :, :], exp_f)
                    for h2 in (0, 1):
                        t = 2 * p + h2
                        if t >= Wp - 8:
                            nc.vector.tensor_mul(pt[:, h2, :], pt[:, h2, :],
                                                 mt[:, t - (Wp - 8), :])
                    if p == 0:
                        nc.vector.tensor_copy(pacc[:, :, :], pt[:, :, :])
                    else:
                        nc.vector.tensor_add(pacc[:, :, :], pacc[:, :, :], pt[:, :, :])

                for p in range(P):
                    scores(p)
                    if p >= 1:
                        av(p - 1)
                        pts.pop(p - 1)
                    exp_mask_acc(p)
                    # evenly paced filler consumption across the slot
                    target = -(-q0 * (p + 1) // P)
                    while q0 - len(fillq) < target and fillq:
                        fillq.pop(0)()
                av(P - 1)
                pts.pop(P - 1)

                # epilogue, pipelined over column halves
                pacc1 = epp.tile([128, 512], dt.float16, tag="pacc1", name=f"pacc1{s}")
                lall = epp.tile([128, 512], dt.float32, tag="lall", name=f"lall{s}")
                rb = epp.tile([128, 512], dt.float32, tag="rb", name=f"rb{s}")
                tmpo = epp.tile([128, 512], dt.float16, tag="tmpo", name=f"tmpo{s}")
                onrm = epp.tile([128, 512], dt.float16, tag="onrm", name=f"onrm{s}")
                for hh in (slice(0, 256), slice(256, 512)):
                    nc.vector.tensor_add(pacc1[:, hh], pacc[:, 0, hh], pacc[:, 1, hh])
                    nc.gpsimd.partition_all_reduce(lall[:, hh], pacc1[:, hh], 128,
                                                   bass_isa.ReduceOp.add)
                    nc.vector.reciprocal(rb[:, hh], lall[:, hh])
                    nc.vector.tensor_mul(tmpo[:, hh], po[:, hh], rb[:, hh])
                    nc.vector.tensor_scalar_add(onrm[:, hh], tmpo[:, hh], bv_sb[:, :])
                    # out stays [H, q] in DRAM; host transposes. Last slot uses
                    # the (by then idle) SP HWDGE queue for lower latency.
                    eng = nc.sync if s == 0 else nc.gpsimd
                    eng.dma_start(out=out_d.ap()[:, 512 * s + hh.start:512 * s + hh.stop],
                                  in_=onrm[:, hh])
                if dbg and s == 3:
                    nc.sync.dma_start(out=dbg["kT0"].ap(), in_=kTs[0][:, :])
                    nc.sync.dma_start(out=dbg["vt0"].ap(), in_=vt4[0][:, :, :])
                    nc.sync.dma_start(out=dbg["qT3"].ap(), in_=qTs[3][:, :])
                    nc.sync.dma_start(out=dbg["pacc3"].ap(), in_=pacc1[:, :])
                    nc.sync.dma_start(out=dbg["lall3"].ap(), in_=lall[:, :])
                    nc.sync.dma_start(out=dbg["po3"].ap(), in_=tmpo[:, :])

            # ================= phase schedule =================
            dma_et(1, nc.sync)
            dma_mask(3, nc.sync)
            for u in kv_units(0):
                u()
            qproj(3)
            dma_et(2, nc.sync)
            for u in kv_units(1):
                u()

            fillq = []
            slot(3, fillq)

            # kv2's et lands mid-slot3; run it directly after
            for u in kv_units(2):
                u()
            qproj(2)
            dma_et(3, nc.sync)
            dma_mask(2, nc.sync)
            fillq += kv_units(3)
            dma_et(4, nc.sync)
            fillq += kv_units(4)
            dma_et(5, nc.sync)
            dma_mask(1, nc.sync)
            slot(2, fillq)

            while fillq:
                fillq.pop(0)()
            qproj(1)
            fillq += kv_units(5)
            dma_et(6, nc.sync)
            fillq += kv_units(6)
            dma_et(7, nc.sync)
            dma_mask(0, nc.sync)
            slot(1, fillq)

            while fillq:
                fillq.pop(0)()
            qproj(0)
            fillq += kv_units(7)
            slot(0, fillq)

    nc.compile()
    return nc


def _build_maskblk(parity):
    m = np.zeros((QB, 128, 8, 512), np.float32)
    kk = np.arange(128)[:, None]
    qq = np.arange(512)[None, :]
    pi = PI[parity]
    for s, j in enumerate(BLOCKS[parity]):
        Wp = PROG[s]
        for i in range(8):
            t = Wp - 8 + i
            d = 4 * (pi[t // 4] - j) + t % 4
            m[s, :, i, :] = ((qq - 128 * d) >= kk)
    return m.astype(FP16)


def _rearrange_w(w):
    # [E, H] -> [128, EC*H] with chunk-major free dim
    return np.ascontiguousarray(
        w.reshape(EC, 128, H).transpose(1, 0, 2).reshape(128, EC * H)).astype(BF16)


def kernel(embds, Wq, bq, Wk, bk, Wv, bv):
    embds = np.asarray(embds, F32)
    Wq = np.asarray(Wq, F32); bq = np.asarray(bq, F32)
    Wk = np.asarray(Wk, F32)
    Wv = np.asarray(Wv, F32); bv = np.asarray(bv, F32)

    if "nc" not in _CACHE:
        _CACHE["nc"] = _build_program()
    nc = _CACHE["nc"]

    scale = F32(1.0 / np.sqrt(H))
    wq_h = _rearrange_w(Wq * scale)
    wk_h = _rearrange_w(Wk)
    wv_h = _rearrange_w(Wv)
    bq_h = (bq * scale).astype(F32).reshape(H, 1)
    bv_h = bv.astype(F32).reshape(H, 1)
    masks = {p: _build_maskblk(p) for p in (0, 1)}

    embT = {b: np.ascontiguousarray(embds[b].T).astype(BF16) for b in range(B)}
    # per-parity key-block permutation of the embedding columns (see PI)
    embTp = {}
    for b in range(B):
        for parity in (0, 1):
            et = embT[b]
            embTp[(b, parity)] = np.ascontiguousarray(
                np.concatenate([et[:, 512 * r:512 * (r + 1)] for r in PI[parity]], axis=1))

    in_maps = []
    for c in range(NCORES):
        b, parity = c // 2, c % 2
        in_maps.append({
            "embT": embTp[(b, parity)],
            "wq": wq_h, "wk": wk_h, "wv": wv_h,
            "bq": bq_h, "bv": bv_h,
            "maskblk": masks[parity],
        })

    res = run_bass_kernel_spmd(nc, in_maps, list(range(NCORES)))
    if os.environ.get("KDEBUG"):
        _CACHE["dbg"] = res.results[0]

    out = np.empty((B, S, H), F32)
    for c in range(NCORES):
        b, parity = c // 2, c % 2
        oc = res.results[c]["outT"].astype(F32).T
        for s, j in enumerate(BLOCKS[parity]):
            out[b, 512 * j:512 * (j + 1)] = oc[512 * s:512 * (s + 1)]
    return out


# revision 46
# speedup vs baseline: 1.4401x; 1.0073x over previous
"""Causal single-head attention (B=4, S=4096, E=1024, H=128) on 8 TRN2 NeuronCores.

Sharding: 8 cores = 4 batches x 2 sequence shards. Each core owns 4 query
blocks of 512 rows of one batch (parity split {7,5,2,0} / {6,4,3,1}); causal
work is padded to a uniform program of [32,24,16,8] k-tiles per slot so all 8
cores run one SPMD program. Per-core inputs (gathered Q columns + causal
masks) encode which q-blocks a core owns.

Schedule: K/V projection blocks are interleaved INTO the attention slots as
PE filler so the tensor engine never idles while the ACT engine runs exp.
Slots are processed smallest-first [8,16,24,32] so early slots only need the
first key blocks. V is projected directly in [keys, H] layout (lhsT=embT
slice, rhs=Wv) so no PE transposes are needed; K-bias is dropped (softmax is
invariant to per-query constants) and V-bias is folded into the output
epilogue. Softmax denominator: exp pairs accumulated in fp16 on DVE, reduced
across partitions on GPSIMD, reciprocal on DVE. Output is normalized to fp16,
transposed by the DMA XBAR, and written out as fp16 (host upcasts).
"""

import os
import numpy as np
import ml_dtypes

import concourse.bacc as bacc
import concourse.bass_isa as bass_isa
import concourse.mybir as mybir
import concourse.tile as tile
from concourse.bass_utils import run_bass_kernel_spmd

BF16 = ml_dtypes.bfloat16
FP16 = np.float16
FP8 = ml_dtypes.float8_e4m3
F32 = np.float32

B, S, E, H = 4, 4096, 1024, 128
NCORES = 8
PROG = [32, 24, 16, 8]                       # program k-tile count per slot
BLOCKS = {0: [7, 5, 2, 0], 1: [6, 4, 3, 1]}  # parity -> owned q-block ids
# per-parity key-block permutation: position p of the core's embT holds real
# block PI[parity][p]. Chosen so each slot's q-block sits at the fixed
# position POS[s] (so Q-projection reads the et tiles; no separate q gather)
# while every slot's allowed key set remains a prefix of positions.
PI = {0: [0, 1, 2, 3, 5, 4, 7, 6], 1: [1, 0, 3, 2, 4, 5, 6, 7]}
POS = {3: 0, 2: 2, 1: 4, 0: 6}               # slot -> position of its q-block
EC = E // 128                                 # 8 contraction chunks
SB = S // 512                                 # 8 key blocks of 512
QB = 4                                        # q-blocks (slots) per core
QLEN = QB * 512                               # 2048 q rows per core

_CACHE = {}


def _build_program():
    dt = mybir.dt
    nc = bacc.Bacc("TRN2", target_bir_lowering=False, debug=False, num_devices=NCORES)

    embT_d = nc.dram_tensor("embT", [E, S], dt.bfloat16, kind="ExternalInput")
    # weights pre-arranged on host to [128, EC*H] (partition-major chunks)
    wq_d = nc.dram_tensor("wq", [128, EC * H], dt.bfloat16, kind="ExternalInput")
    wk_d = nc.dram_tensor("wk", [128, EC * H], dt.bfloat16, kind="ExternalInput")
    wv_d = nc.dram_tensor("wv", [128, EC * H], dt.bfloat16, kind="ExternalInput")
    bq_d = nc.dram_tensor("bq", [H, 1], dt.float32, kind="ExternalInput")
    bv_d = nc.dram_tensor("bv", [H, 1], dt.float32, kind="ExternalInput")
    mask_d = nc.dram_tensor("maskblk", [QB, 128, 8, 512], dt.float16, kind="ExternalInput")
    # output stays transposed [H, QLEN]; host transposes (part of unshard)
    out_d = nc.dram_tensor("outT", [H, QLEN], dt.float16, kind="ExternalOutput")
    dbg = {}
    if os.environ.get("KDEBUG"):
        dbg["kT0"] = nc.dram_tensor("dbg_kT0", [128, 512], dt.bfloat16, kind="ExternalOutput")
        dbg["vt0"] = nc.dram_tensor("dbg_vt0", [128, 4, 128], dt.float16, kind="ExternalOutput")
        dbg["qT3"] = nc.dram_tensor("dbg_qT3", [128, 512], dt.bfloat16, kind="ExternalOutput")
        dbg["pacc3"] = nc.dram_tensor("dbg_pacc3", [128, 512], dt.float16, kind="ExternalOutput")
        dbg["lall3"] = nc.dram_tensor("dbg_lall3", [128, 512], dt.float32, kind="ExternalOutput")
        dbg["po3"] = nc.dram_tensor("dbg_po3", [128, 512], dt.float16, kind="ExternalOutput")

    ident_f = mybir.ActivationFunctionType.Identity
    exp_f = mybir.ActivationFunctionType.Exp

    with tile.TileContext(nc) as tc:
        with tc.tile_pool(name="singles", bufs=1) as singles, \
             tc.tile_pool(name="et", bufs=4) as etp, \
             tc.tile_pool(name="pk", bufs=1, space="PSUM") as pkp, \
             tc.tile_pool(name="pv", bufs=1, space="PSUM") as pvp, \
             tc.tile_pool(name="ps", bufs=2, space="PSUM") as psp, \
             tc.tile_pool(name="po", bufs=2, space="PSUM") as pop, \
             tc.tile_pool(name="mask", bufs=4) as mkp, \
             tc.tile_pool(name="pt", bufs=3) as ptp, \
             tc.tile_pool(name="pacc", bufs=2) as pacp, \
             tc.tile_pool(name="ep", bufs=2) as epp:

            # ---- constant tiles ----
            w_sb = {}
            for name in ("k", "v", "q"):
                w_sb[name] = singles.tile([128, EC, H], dt.bfloat16, tag=f"w{name}", name=f"w{name}")
            bq_sb = singles.tile([H, 1], dt.float32, tag="bq")
            bv_sb = singles.tile([H, 1], dt.float32, tag="bv")
            kTs = [singles.tile([128, 512], dt.bfloat16, tag=f"kT{i}", name=f"kT{i}") for i in range(SB)]
            vt4 = [singles.tile([128, 4, 128], dt.float16, tag=f"v{i}", name=f"v{i}") for i in range(SB)]
            qTs = [singles.tile([128, 512], dt.bfloat16, tag=f"qT{i}", name=f"qT{i}") for i in range(QB)]

            ets = {}
            mts = {}

            # ---- startup DMA burst (need-ordered; sync + scalar alternate) ----
            nc.sync.dma_start(out=w_sb["k"][:, 0, :], in_=wk_d.ap()[:, 0:H])
            et0 = etp.tile([128, EC, 512], dt.bfloat16, tag="et", name="et0")
            ets[0] = et0
            nc.scalar.dma_start(
                out=et0[:, 0:2, :],
                in_=embT_d.ap().rearrange("(c p) s -> p c s", p=128)[:, 0:2, 0:512])
            nc.sync.dma_start(out=w_sb["k"][:, 1:EC, :],
                              in_=wk_d.ap()[:, H:EC * H].rearrange("p (c h) -> p c h", h=H))
            nc.scalar.dma_start(
                out=et0[:, 2:4, :],
                in_=embT_d.ap().rearrange("(c p) s -> p c s", p=128)[:, 2:4, 0:512])
            nc.sync.dma_start(out=w_sb["v"][:, :, :],
                              in_=wv_d.ap().rearrange("p (c h) -> p c h", h=H))
            nc.scalar.dma_start(
                out=et0[:, 4:6, :],
                in_=embT_d.ap().rearrange("(c p) s -> p c s", p=128)[:, 4:6, 0:512])
            nc.sync.dma_start(out=bq_sb[:, :], in_=bq_d.ap())
            nc.scalar.dma_start(
                out=et0[:, 6:8, :],
                in_=embT_d.ap().rearrange("(c p) s -> p c s", p=128)[:, 6:8, 0:512])
            nc.sync.dma_start(out=bv_sb[:, :], in_=bv_d.ap())
            nc.scalar.dma_start(out=w_sb["q"][:, :, :],
                                in_=wq_d.ap().rearrange("p (c h) -> p c h", h=H))
            # remaining input DMAs in strict need-order (the DMA pipe is the
            # prologue bottleneck; transfers execute in HWDGE-issue order)

            def dma_et(b, eng):
                t = etp.tile([128, EC, 512], dt.bfloat16, tag="et", name=f"et{b}")
                ets[b] = t
                eng.dma_start(
                    out=t[:, :, :],
                    in_=embT_d.ap().rearrange("(c p) s -> p c s", p=128)[:, :, 512 * b:512 * (b + 1)])

            def dma_mask(s, eng):
                t = mkp.tile([128, 8, 512], dt.float16, tag="mt", name=f"mt{s}")
                mts[s] = t
                eng.dma_start(out=t[:, :, :], in_=mask_d.ap()[s])

            # ---- kv block emitters: 16 PE units (8 K-chunks, 8 V-chunks) ----
            def kv_units(b):
                units = []
                et = ets[b]
                psk = pkp.tile([128, 512], dt.float32, tag="psk", name=f"psk{b}")
                psv = pvp.tile([128, 4, 128], dt.float32, tag="psv", name=f"psv{b}")

                def k_chunk(c):
                    def emit():
                        nc.tensor.matmul(psk[:, :], lhsT=w_sb["k"][:, c, :], rhs=et[:, c, :],
                                         start=(c == 0), stop=(c == EC - 1))
                        if c == EC - 1:
                            nc.vector.tensor_copy(kTs[b][:, :], psk[:, :])
                    return emit

                def v_unit(u):
                    # one full accumulation group per bank region; groups in the
                    # same PSUM bank must not interleave (codegen breaks)
                    def emit():
                        for c in range(EC):
                            nc.tensor.matmul(psv[:, u, :],
                                             lhsT=et[:, c, 128 * u:128 * (u + 1)],
                                             rhs=w_sb["v"][:, c, :],
                                             start=(c == 0), stop=(c == EC - 1))
                        if u == 3:
                            nc.vector.tensor_copy(vt4[b][:, :, :], psv[:, :, :])
                    return emit

                for c in range(EC):
                    units.append(k_chunk(c))
                for u in range(4):
                    units.append(v_unit(u))
                return units

            def qproj(s):
                etq = ets[POS[s]]
                psq = pop.tile([128, 512], dt.float32, tag="po", name=f"psq{s}")
                for c in range(EC):
                    nc.tensor.matmul(psq[:, :], lhsT=w_sb["q"][:, c, :], rhs=etq[:, c, :],
                                     start=(c == 0), stop=(c == EC - 1))
                nc.scalar.activation(qTs[s][:, :], psq[:, :], ident_f, bias=bq_sb[:, :])

            # ---- attention slot with PE filler consumption ----
            def slot(s, fillq):
                Wp = PROG[s]
                P = Wp // 2
                mt = mts[s]
                po = pop.tile([128, 512], dt.float32, tag="po", name=f"po{s}")
                pacc = pacp.tile([128, 2, 512], dt.float16, tag="pacc", name=f"pacc{s}")
                pss, pts = {}, {}
                q0 = len(fillq)

                def scores(p):
                    ps = psp.tile([128, 2, 512], dt.float32, tag="ps", name=f"ps{s}_{p}")
                    pss[p] = ps
                    for h2 in (0, 1):
                        t = 2 * p + h2
                        nc.tensor.matmul(ps[:, h2, :],
                                         lhsT=kTs[t // 4][:, 128 * (t % 4):128 * (t % 4 + 1)],
                                         rhs=qTs[s][:, :], start=True, stop=True)

                def av(p):
                    pt = pts[p]
                    for h2 in (0, 1):
                        t = 2 * p + h2
                        nc.tensor.matmul(po[:, :], lhsT=vt4[t // 4][:, t % 4, :],
                                         rhs=pt[:, h2, :],
                                         start=(t == 0), stop=(t == Wp - 1))

                def exp_mask_acc(p):
                    ps = pss.pop(p)
                    pt = ptp.tile([128, 2, 512], dt.float16, tag="pt", name=f"pt{s}_{p}")
                    pts[p] = pt
                    nc.scalar.activation(pt[:, :, :], ps[:, :, :], exp_f)
                    for h2 in (0, 1):
                        t = 2 * p + h2
                        if t >= Wp - 8:
                            nc.vector.tensor_mul(pt[:, h2, :], pt[:, h2, :],
                                                 mt[:, t - (Wp - 8), :])
                    if p == 0:
                        nc.vector.tensor_copy(pacc[:, :, :], pt[:, :, :])
                    else:
                        nc.vector.tensor_add(pacc[:, :, :], pacc[:, :, :], pt[:, :, :])

                for p in range(P):
                    scores(p)
                    if p >= 1:
                        av(p - 1)
                        pts.pop(p - 1)
                    exp_mask_acc(p)
                    # evenly paced filler consumption across the slot
                    target = -(-q0 * (p + 1) // P)
                    while q0 - len(fillq) < target and fillq:
                        fillq.pop(0)()
                av(P - 1)
                pts.pop(P - 1)

                # epilogue, pipelined over column halves
                pacc1 = epp.tile([128, 512], dt.float16, tag="pacc1", name=f"pacc1{s}")
                lall = epp.tile([128, 512], dt.float32, tag="lall", name=f"lall{s}")
                rb = epp.tile([128, 512], dt.float32, tag="rb", name=f"rb{s}")
                tmpo = epp.tile([128, 512], dt.float16, tag="tmpo", name=f"tmpo{s}")
                onrm = epp.tile([128, 512], dt.float16, tag="onrm", name=f"onrm{s}")
                for hh in (slice(0, 256), slice(256, 512)):
                    nc.vector.tensor_add(pacc1[:, hh], pacc[:, 0, hh], pacc[:, 1, hh])
                    nc.gpsimd.partition_all_reduce(lall[:, hh], pacc1[:, hh], 128,
                                                   bass_isa.ReduceOp.add)
                    nc.vector.reciprocal(rb[:, hh], lall[:, hh])
                    nc.vector.tensor_mul(tmpo[:, hh], po[:, hh], rb[:, hh])
                    nc.vector.tensor_scalar_add(onrm[:, hh], tmpo[:, hh], bv_sb[:, :])
                    # out stays [H, q] in DRAM; host transposes. Last slot uses
                    # the (by then idle) SP HWDGE queue for lower latency.
                    eng = nc.sync if s == 0 else nc.gpsimd
                    eng.dma_start(out=out_d.ap()[:, 512 * s + hh.start:512 * s + hh.stop],
                                  in_=onrm[:, hh])
                if dbg and s == 3:
                    nc.sync.dma_start(out=dbg["kT0"].ap(), in_=kTs[0][:, :])
                    nc.sync.dma_start(out=dbg["vt0"].ap(), in_=vt4[0][:, :, :])
                    nc.sync.dma_start(out=dbg["qT3"].ap(), in_=qTs[3][:, :])
                    nc.sync.dma_start(out=dbg["pacc3"].ap(), in_=pacc1[:, :])
                    nc.sync.dma_start(out=dbg["lall3"].ap(), in_=lall[:, :])
                    nc.sync.dma_start(out=dbg["po3"].ap(), in_=tmpo[:, :])

            # ================= phase schedule =================
            dma_et(1, nc.sync)
            dma_mask(3, nc.sync)
            for u in kv_units(0):
                u()
            qproj(3)
            dma_et(2, nc.sync)
            for u in kv_units(1):
                u()

            fillq = []
            slot(3, fillq)

            # kv2's et lands mid-slot3; run it directly after
            for u in kv_units(2):
                u()
            qproj(2)
            dma_et(3, nc.sync)
            dma_mask(2, nc.sync)
            fillq += kv_units(3)
            dma_et(4, nc.sync)
            fillq += kv_units(4)
            dma_et(5, nc.sync)
            dma_mask(1, nc.sync)
            slot(2, fillq)

            while fillq:
                fillq.pop(0)()
            qproj(1)
            fillq += kv_units(5)
            dma_et(6, nc.sync)
            fillq += kv_units(6)
            dma_et(7, nc.sync)
            dma_mask(0, nc.sync)
            slot(1, fillq)

            while fillq:
                fillq.pop(0)()
            qproj(0)
            fillq += kv_units(7)
            slot(0, fillq)

    nc.compile()
    return nc


def _build_maskblk(parity):
    m = np.zeros((QB, 128, 8, 512), np.float32)
    kk = np.arange(128)[:, None]
    qq = np.arange(512)[None, :]
    pi = PI[parity]
    for s, j in enumerate(BLOCKS[parity]):
        Wp = PROG[s]
        for i in range(8):
            t = Wp - 8 + i
            d = 4 * (pi[t // 4] - j) + t % 4
            m[s, :, i, :] = ((qq - 128 * d) >= kk)
    return m.astype(FP16)


def _rearrange_w(w):
    # [E, H] -> [128, EC*H] with chunk-major free dim
    return np.ascontiguousarray(
        w.reshape(EC, 128, H).transpose(1, 0, 2).reshape(128, EC * H)).astype(BF16)


def kernel(embds, Wq, bq, Wk, bk, Wv, bv):
    embds = np.asarray(embds, F32)
    Wq = np.asarray(Wq, F32); bq = np.asarray(bq, F32)
    Wk = np.asarray(Wk, F32)
    Wv = np.asarray(Wv, F32); bv = np.asarray(bv, F32)

    if "nc" not in _CACHE:
        _CACHE["nc"] = _build_program()
    nc = _CACHE["nc"]

    scale = F32(1.0 / np.sqrt(H))
    wq_h = _rearrange_w(Wq * scale)
    wk_h = _rearrange_w(Wk)
    wv_h = _rearrange_w(Wv)
    bq_h = (bq * scale).astype(F32).reshape(H, 1)
    bv_h = bv.astype(F32).reshape(H, 1)
    masks = {p: _build_maskblk(p) for p in (0, 1)}

    embT = {b: np.ascontiguousarray(embds[b].T).astype(BF16) for b in range(B)}
    # per-parity key-block permutation of the embedding columns (see PI)
    embTp = {}
    for b in range(B):
        for parity in (0, 1):
            et = embT[b]
            embTp[(b, parity)] = np.ascontiguousarray(
                np.concatenate([et[:, 512 * r:512 * (r + 1)] for r in PI[parity]], axis=1))

    in_maps = []
    for c in range(NCORES):
        b, parity = c // 2, c % 2
        in_maps.append({
            "embT": embTp[(b, parity)],
            "wq": wq_h, "wk": wk_h, "wv": wv_h,
            "bq": bq_h, "bv": bv_h,
            "maskblk": masks[parity],
        })

    res = run_bass_kernel_spmd(nc, in_maps, list(range(NCORES)))
    if os.environ.get("KDEBUG"):
        _CACHE["dbg"] = res.results[0]

    out = np.empty((B, S, H), F32)
    for c in range(NCORES):
        b, parity = c // 2, c % 2
        oc = res.results[c]["outT"].astype(F32).T
        for s, j in enumerate(BLOCKS[parity]):
            out[b, 512 * j:512 * (j + 1)] = oc[512 * s:512 * (s + 1)]
    return out


# revision 58
# speedup vs baseline: 1.4653x; 1.0175x over previous
"""Causal single-head attention (B=4, S=4096, E=1024, H=128) on 8 TRN2 NeuronCores.

Sharding: 8 cores = 4 batches x 2 sequence shards. Each core owns 4 query
blocks of 512 rows of one batch (parity split {7,5,2,0} / {6,4,3,1}); causal
work is padded to a uniform program of [32,24,16,8] k-tiles per slot so all 8
cores run one SPMD program. Per-core inputs (gathered Q columns + causal
masks) encode which q-blocks a core owns.

Schedule: K/V projection blocks are interleaved INTO the attention slots as
PE filler so the tensor engine never idles while the ACT engine runs exp.
Slots are processed smallest-first [8,16,24,32] so early slots only need the
first key blocks. V is projected directly in [keys, H] layout (lhsT=embT
slice, rhs=Wv) so no PE transposes are needed; K-bias is dropped (softmax is
invariant to per-query constants) and V-bias is folded into the output
epilogue. Softmax denominator: exp pairs accumulated in fp16 on DVE, reduced
across partitions on GPSIMD, reciprocal on DVE. Output is normalized to fp16,
transposed by the DMA XBAR, and written out as fp16 (host upcasts).
"""

import os
import numpy as np
import ml_dtypes

import concourse.bacc as bacc
import concourse.bass_isa as bass_isa
import concourse.mybir as mybir
import concourse.tile as tile
from concourse.bass_utils import run_bass_kernel_spmd

BF16 = ml_dtypes.bfloat16
FP16 = np.float16
FP8 = ml_dtypes.float8_e4m3
F32 = np.float32

B, S, E, H = 4, 4096, 1024, 128
NCORES = 8
PROG = [32, 24, 16, 8]                       # program k-tile count per slot
BLOCKS = {0: [7, 5, 2, 0], 1: [6, 4, 3, 1]}  # parity -> owned q-block ids
# per-parity key-block permutation: position p of the core's embT holds real
# block PI[parity][p]. Chosen so each slot's q-block sits at the fixed
# position POS[s] (so Q-projection reads the et tiles; no separate q gather)
# while every slot's allowed key set remains a prefix of positions.
PI = {0: [0, 1, 2, 3, 5, 4, 7, 6], 1: [1, 0, 3, 2, 4, 5, 6, 7]}
POS = {3: 0, 2: 2, 1: 4, 0: 6}               # slot -> position of its q-block
EC = E // 128                                 # 8 contraction chunks
SB = S // 512                                 # 8 key blocks of 512
QB = 4                                        # q-blocks (slots) per core
QLEN = QB * 512                               # 2048 q rows per core

_CACHE = {}


def _build_program():
    dt = mybir.dt
    nc = bacc.Bacc("TRN2", target_bir_lowering=False, debug=False, num_devices=NCORES)

    embT_d = nc.dram_tensor("embT", [E, S], dt.bfloat16, kind="ExternalInput")
    # weights pre-arranged on host to [128, EC*H] (partition-major chunks)
    wq_d = nc.dram_tensor("wq", [128, EC * H], dt.bfloat16, kind="ExternalInput")
    wk_d = nc.dram_tensor("wk", [128, EC * H], dt.bfloat16, kind="ExternalInput")
    wv_d = nc.dram_tensor("wv", [128, EC * H], dt.bfloat16, kind="ExternalInput")
    bq_d = nc.dram_tensor("bq", [H, 1], dt.float32, kind="ExternalInput")
    bv_d = nc.dram_tensor("bv", [H, 1], dt.float32, kind="ExternalInput")
    mask_d = nc.dram_tensor("maskblk", [QB, 128, 8, 512], dt.float16, kind="ExternalInput")
    # output stays transposed [H, QLEN]; host transposes (part of unshard)
    out_d = nc.dram_tensor("outT", [H, QLEN], dt.float16, kind="ExternalOutput")
    dbg = {}
    if os.environ.get("KDEBUG"):
        dbg["kT0"] = nc.dram_tensor("dbg_kT0", [128, 512], dt.bfloat16, kind="ExternalOutput")
        dbg["vt0"] = nc.dram_tensor("dbg_vt0", [128, 4, 128], dt.float16, kind="ExternalOutput")
        dbg["qT3"] = nc.dram_tensor("dbg_qT3", [128, 512], dt.bfloat16, kind="ExternalOutput")
        dbg["pacc3"] = nc.dram_tensor("dbg_pacc3", [128, 512], dt.float16, kind="ExternalOutput")
        dbg["lall3"] = nc.dram_tensor("dbg_lall3", [128, 512], dt.float32, kind="ExternalOutput")
        dbg["po3"] = nc.dram_tensor("dbg_po3", [128, 512], dt.float16, kind="ExternalOutput")

    ident_f = mybir.ActivationFunctionType.Identity
    exp_f = mybir.ActivationFunctionType.Exp

    with tile.TileContext(nc) as tc:
        with tc.tile_pool(name="singles", bufs=1) as singles, \
             tc.tile_pool(name="et", bufs=4) as etp, \
             tc.tile_pool(name="pk", bufs=1, space="PSUM") as pkp, \
             tc.tile_pool(name="pv", bufs=1, space="PSUM") as pvp, \
             tc.tile_pool(name="ps", bufs=2, space="PSUM") as psp, \
             tc.tile_pool(name="po", bufs=2, space="PSUM") as pop, \
             tc.tile_pool(name="mask", bufs=4) as mkp, \
             tc.tile_pool(name="pt", bufs=3) as ptp, \
             tc.tile_pool(name="pacc", bufs=2) as pacp, \
             tc.tile_pool(name="ep", bufs=2) as epp:

            # ---- constant tiles ----
            w_sb = {}
            for name in ("k", "v", "q"):
                w_sb[name] = singles.tile([128, EC, H], dt.bfloat16, tag=f"w{name}", name=f"w{name}")
            bq_sb = singles.tile([H, 1], dt.float32, tag="bq")
            bv_sb = singles.tile([H, 1], dt.float32, tag="bv")
            kTs = [singles.tile([128, 512], dt.bfloat16, tag=f"kT{i}", name=f"kT{i}") for i in range(SB)]
            vt4 = [singles.tile([128, 4, 128], dt.float16, tag=f"v{i}", name=f"v{i}") for i in range(SB)]
            qTs = [singles.tile([128, 512], dt.bfloat16, tag=f"qT{i}", name=f"qT{i}") for i in range(QB)]

            ets = {}
            mts = {}

            # ---- startup DMA burst (need-ordered; sync + scalar alternate) ----
            nc.sync.dma_start(out=w_sb["k"][:, 0, :], in_=wk_d.ap()[:, 0:H])
            et0 = etp.tile([128, EC, 512], dt.bfloat16, tag="et", name="et0")
            ets[0] = et0
            nc.scalar.dma_start(
                out=et0[:, 0:2, :],
                in_=embT_d.ap().rearrange("(c p) s -> p c s", p=128)[:, 0:2, 0:512])
            nc.sync.dma_start(out=w_sb["k"][:, 1:EC, :],
                              in_=wk_d.ap()[:, H:EC * H].rearrange("p (c h) -> p c h", h=H))
            nc.scalar.dma_start(
                out=et0[:, 2:4, :],
                in_=embT_d.ap().rearrange("(c p) s -> p c s", p=128)[:, 2:4, 0:512])
            nc.sync.dma_start(out=w_sb["v"][:, :, :],
                              in_=wv_d.ap().rearrange("p (c h) -> p c h", h=H))
            nc.scalar.dma_start(
                out=et0[:, 4:6, :],
                in_=embT_d.ap().rearrange("(c p) s -> p c s", p=128)[:, 4:6, 0:512])
            nc.sync.dma_start(out=bq_sb[:, :], in_=bq_d.ap())
            nc.scalar.dma_start(
                out=et0[:, 6:8, :],
                in_=embT_d.ap().rearrange("(c p) s -> p c s", p=128)[:, 6:8, 0:512])
            nc.sync.dma_start(out=bv_sb[:, :], in_=bv_d.ap())
            nc.scalar.dma_start(out=w_sb["q"][:, :, :],
                                in_=wq_d.ap().rearrange("p (c h) -> p c h", h=H))
            # remaining input DMAs in strict need-order (the DMA pipe is the
            # prologue bottleneck; transfers execute in HWDGE-issue order)

            def dma_et(b, eng):
                t = etp.tile([128, EC, 512], dt.bfloat16, tag="et", name=f"et{b}")
                ets[b] = t
                eng.dma_start(
                    out=t[:, :, :],
                    in_=embT_d.ap().rearrange("(c p) s -> p c s", p=128)[:, :, 512 * b:512 * (b + 1)])

            def dma_mask(s, eng):
                t = mkp.tile([128, 8, 512], dt.float16, tag="mt", name=f"mt{s}")
                mts[s] = t
                eng.dma_start(out=t[:, :, :], in_=mask_d.ap()[s])

            # ---- kv block emitters: 16 PE units (8 K-chunks, 8 V-chunks) ----
            def kv_units(b):
                units = []
                et = ets[b]
                psk = pkp.tile([128, 512], dt.float32, tag="psk", name=f"psk{b}")
                psv = pvp.tile([128, 4, 128], dt.float32, tag="psv", name=f"psv{b}")

                def k_chunk(c):
                    def emit():
                        nc.tensor.matmul(psk[:, :], lhsT=w_sb["k"][:, c, :], rhs=et[:, c, :],
                                         start=(c == 0), stop=(c == EC - 1))
                        if c == EC - 1:
                            # ACT has slack; keep DVE for masks/pacc
                            nc.scalar.activation(kTs[b][:, :], psk[:, :],
                                                 mybir.ActivationFunctionType.Copy)
                    return emit

                def v_unit(u):
                    # one full accumulation group per bank region; groups in the
                    # same PSUM bank must not interleave (codegen breaks)
                    def emit():
                        for c in range(EC):
                            nc.tensor.matmul(psv[:, u, :],
                                             lhsT=et[:, c, 128 * u:128 * (u + 1)],
                                             rhs=w_sb["v"][:, c, :],
                                             start=(c == 0), stop=(c == EC - 1))
                        if u == 3:
                            nc.vector.tensor_copy(vt4[b][:, :, :], psv[:, :, :])
                    return emit

                # deadline: all of block b's units must be emitted before the
                # slot iteration that first reads kTs[b]/vt4[b] (pair 2b)
                dl = 2 * b - 1
                for c in range(EC):
                    units.append((dl, k_chunk(c)))
                for u in range(4):
                    units.append((dl, v_unit(u)))
                return units

            def qproj(s):
                etq = ets[POS[s]]
                psq = pop.tile([128, 512], dt.float32, tag="po", name=f"psq{s}")
                for c in range(EC):
                    nc.tensor.matmul(psq[:, :], lhsT=w_sb["q"][:, c, :], rhs=etq[:, c, :],
                                     start=(c == 0), stop=(c == EC - 1))
                nc.scalar.activation(qTs[s][:, :], psq[:, :], ident_f, bias=bq_sb[:, :])

            # ---- attention slot with PE filler consumption ----
            def slot(s, fillq):
                Wp = PROG[s]
                P = Wp // 2
                mt = mts[s]
                po = pop.tile([128, 512], dt.float32, tag="po", name=f"po{s}")
                pacc = pacp.tile([128, 2, 512], dt.float16, tag="pacc", name=f"pacc{s}")
                pss, pts = {}, {}
                q0 = len(fillq)

                def scores(p):
                    ps = psp.tile([128, 2, 512], dt.float32, tag="ps", name=f"ps{s}_{p}")
                    pss[p] = ps
                    for h2 in (0, 1):
                        t = 2 * p + h2
                        nc.tensor.matmul(ps[:, h2, :],
                                         lhsT=kTs[t // 4][:, 128 * (t % 4):128 * (t % 4 + 1)],
                                         rhs=qTs[s][:, :], start=True, stop=True)

                def av(p):
                    pt = pts[p]
                    for h2 in (0, 1):
                        t = 2 * p + h2
                        nc.tensor.matmul(po[:, :], lhsT=vt4[t // 4][:, t % 4, :],
                                         rhs=pt[:, h2, :],
                                         start=(t == 0), stop=(t == Wp - 1))

                def exp_mask_acc(p):
                    ps = pss.pop(p)
                    pt = ptp.tile([128, 2, 512], dt.float16, tag="pt", name=f"pt{s}_{p}")
                    pts[p] = pt
                    nc.scalar.activation(pt[:, :, :], ps[:, :, :], exp_f)
                    for h2 in (0, 1):
                        t = 2 * p + h2
                        if t >= Wp - 8:
                            nc.vector.tensor_mul(pt[:, h2, :], pt[:, h2, :],
                                                 mt[:, t - (Wp - 8), :])
                    if p == 0:
                        nc.vector.tensor_copy(pacc[:, :, :], pt[:, :, :])
                    else:
                        nc.vector.tensor_add(pacc[:, :, :], pacc[:, :, :], pt[:, :, :])

                for p in range(P):
                    scores(p)
                    if p >= 1:
                        av(p - 1)
                        pts.pop(p - 1)
                    exp_mask_acc(p)
                    # evenly paced filler consumption, but never past a unit's
                    # deadline (the pair that first reads its block's outputs)
                    target = -(-q0 * (p + 1) // P)
                    while fillq and (q0 - len(fillq) < target or fillq[0][0] <= p):
                        fillq.pop(0)[1]()
                av(P - 1)
                pts.pop(P - 1)

                # epilogue, pipelined over column halves
                pacc1 = epp.tile([128, 512], dt.float16, tag="pacc1", name=f"pacc1{s}")
                lall = epp.tile([128, 512], dt.float32, tag="lall", name=f"lall{s}")
                rb = epp.tile([128, 512], dt.float32, tag="rb", name=f"rb{s}")
                tmpo = epp.tile([128, 512], dt.float16, tag="tmpo", name=f"tmpo{s}")
                onrm = epp.tile([128, 512], dt.float16, tag="onrm", name=f"onrm{s}")
                halves = (slice(0, 256), slice(256, 512))
                for hh in halves:
                    nc.vector.tensor_add(pacc1[:, hh], pacc[:, 0, hh], pacc[:, 1, hh])
                for hh in halves:
                    nc.gpsimd.partition_all_reduce(lall[:, hh], pacc1[:, hh], 128,
                                                   bass_isa.ReduceOp.add)
                for hh in halves:
                    nc.vector.reciprocal(rb[:, hh], lall[:, hh])
                for hh in halves:
                    nc.vector.tensor_mul(tmpo[:, hh], po[:, hh], rb[:, hh])
                    nc.vector.tensor_scalar_add(onrm[:, hh], tmpo[:, hh], bv_sb[:, :])
                    # out stays [H, q] in DRAM; host transposes. Last slot uses
                    # the (by then idle) SP HWDGE queue for lower latency.
                    eng = nc.sync if s == 0 else nc.gpsimd
                    eng.dma_start(out=out_d.ap()[:, 512 * s + hh.start:512 * s + hh.stop],
                                  in_=onrm[:, hh])
                if dbg and s == 3:
                    nc.sync.dma_start(out=dbg["kT0"].ap(), in_=kTs[0][:, :])
                    nc.sync.dma_start(out=dbg["vt0"].ap(), in_=vt4[0][:, :, :])
                    nc.sync.dma_start(out=dbg["qT3"].ap(), in_=qTs[3][:, :])
                    nc.sync.dma_start(out=dbg["pacc3"].ap(), in_=pacc1[:, :])
                    nc.sync.dma_start(out=dbg["lall3"].ap(), in_=lall[:, :])
                    nc.sync.dma_start(out=dbg["po3"].ap(), in_=tmpo[:, :])

            # ================= phase schedule =================
            dma_et(1, nc.sync)
            dma_mask(3, nc.sync)
            for _, u in kv_units(0):
                u()
            qproj(3)
            dma_et(2, nc.sync)
            for _, u in kv_units(1):
                u()

            fillq = []
            slot(3, fillq)

            # kv2's et lands mid-slot3; run it directly after
            for _, u in kv_units(2):
                u()
            qproj(2)
            dma_et(3, nc.sync)
            dma_mask(2, nc.sync)
            fillq += kv_units(3)
            dma_et(4, nc.sync)
            dma_et(5, nc.sync)
            dma_mask(1, nc.sync)
            slot(2, fillq)

            while fillq:
                fillq.pop(0)[1]()
            qproj(1)
            fillq += kv_units(4)
            fillq += kv_units(5)
            dma_et(6, nc.sync)
            dma_et(7, nc.sync)
            dma_mask(0, nc.sync)
            slot(1, fillq)

            while fillq:
                fillq.pop(0)[1]()
            qproj(0)
            fillq += kv_units(6)
            fillq += kv_units(7)
            slot(0, fillq)

    nc.compile()
    return nc


def _build_maskblk(parity):
    m = np.zeros((QB, 128, 8, 512), np.float32)
    kk = np.arange(128)[:, None]
    qq = np.arange(512)[None, :]
    pi = PI[parity]
    for s, j in enumerate(BLOCKS[parity]):
        Wp = PROG[s]
        for i in range(8):
            t = Wp - 8 + i
            d = 4 * (pi[t // 4] - j) + t % 4
            m[s, :, i, :] = ((qq - 128 * d) >= kk)
    return m.astype(FP16)


def _rearrange_w(w):
    # [E, H] -> [128, EC*H] with chunk-major free dim
    return np.ascontiguousarray(
        w.reshape(EC, 128, H).transpose(1, 0, 2).reshape(128, EC * H)).astype(BF16)


def kernel(embds, Wq, bq, Wk, bk, Wv, bv):
    embds = np.asarray(embds, F32)
    Wq = np.asarray(Wq, F32); bq = np.asarray(bq, F32)
    Wk = np.asarray(Wk, F32)
    Wv = np.asarray(Wv, F32); bv = np.asarray(bv, F32)

    if "nc" not in _CACHE:
        _CACHE["nc"] = _build_program()
    nc = _CACHE["nc"]

    scale = F32(1.0 / np.sqrt(H))
    wq_h = _rearrange_w(Wq * scale)
    wk_h = _rearrange_w(Wk)
    wv_h = _rearrange_w(Wv)
    bq_h = (bq * scale).astype(F32).reshape(H, 1)
    bv_h = bv.astype(F32).reshape(H, 1)
    masks = {p: _build_maskblk(p) for p in (0, 1)}

    embT = {b: np.ascontiguousarray(embds[b].T).astype(BF16) for b in range(B)}
    # per-parity key-block permutation of the embedding columns (see PI)
    embTp = {}
    for b in range(B):
        for parity in (0, 1):
            et = embT[b]
            embTp[(b, parity)] = np.ascontiguousarray(
                np.concatenate([et[:, 512 * r:512 * (r + 1)] for r in PI[parity]], axis=1))

    in_maps = []
    for c in range(NCORES):
        b, parity = c // 2, c % 2
        in_maps.append({
            "embT": embTp[(b, parity)],
            "wq": wq_h, "wk": wk_h, "wv": wv_h,
            "bq": bq_h, "bv": bv_h,
            "maskblk": masks[parity],
        })

    res = run_bass_kernel_spmd(nc, in_maps, list(range(NCORES)))
    if os.environ.get("KDEBUG"):
        _CACHE["dbg"] = res.results[0]

    out = np.empty((B, S, H), F32)
    for c in range(NCORES):
        b, parity = c // 2, c % 2
        oc = res.results[c]["outT"].astype(F32).T
        for s, j in enumerate(BLOCKS[parity]):
            out[b, 512 * j:512 * (j + 1)] = oc[512 * s:512 * (s + 1)]
    return out
